# revision 48
# baseline (speedup 1.0000x reference)
"""Trainium2 Bass kernel for L4Q quantized linear (LoRA + group fake-quant + GEMM).

Computation (per reference):
    w   = w0 + lora_b @ lora_a                      # [4096, 4096]
    w_q = round(clip(w/s, -8, 7)) * s               # group-wise (groups of 128 along in)
    y   = x @ w_q.T + bias                          # x: [4, 2048, 4096]

Sharding: column-parallel over out_features across 8 cores (512 outs/core).
x is replicated (pre-transposed + fp16-cast on host); each core computes
y[:, :, c*512:(c+1)*512] and the host concatenates.

Numeric strategy:
  - dequant runs in ~fp32-exact arithmetic on-device: the K=16 LoRA delta via
    an exact 2-way fp16 Dekker split of (2^10 * lora) re-scaled back with an
    exact 2^-20 fused multiply; per-group 1/s via NR-refined reciprocal;
    scale/recip rows broadcast to 128 partitions via K=3 bf16 matmuls whose
    three rows are disjoint 8-bit chunks of the fp32 mantissa (exact fp32
    reconstruction in PSUM at bf16 matmul speed); DVE IEEE fp32 elementwise
    with magic-number round-half-even. Quantization decisions match the fp32
    reference to ~1 ulp.
  - the big GEMM runs in fp16 (11-bit mantissa) with fp32 PSUM accumulation;
    y is emitted fp16 (|y| <= ~11, so ~2.5e-3 absolute) and widened on host.

Pipelining: K is split unevenly into 8 + 24 k-tiles. The 8-tile slice is
dequantized up front (short head); the 24-tile slice's dequant (DVE-heavy)
runs interleaved under the first GEMM phase so the PE stays continuously busy
at the warm 2.4 GHz p-state. Phase-0 partial sums (+bias) park in an fp16
SBUF accumulator; phase 1 adds them during its PSUM drain.
"""
import numpy as np

import concourse.bass as bass
import concourse.bacc as bacc
import concourse.mybir as mybir
from concourse.tile import TileContext
from concourse.bass_utils import run_bass_kernel_spmd
from concourse.alu_op_type import AluOpType

F32 = mybir.dt.float32
F16 = mybir.dt.float16
BF16 = mybir.dt.bfloat16
F8 = mybir.dt.float8e4
MAGIC = 12582912.0  # 1.5 * 2**23: forces round-to-nearest-even at integer granularity

N_CORES = 8
IN_F = 4096
OUT_F = 4096
RANK = 16
B, S = 4, 2048
M_TOK = B * S            # 8192 tokens
OUT_SH = OUT_F // N_CORES  # 512 out features per core
GROUP = 128
N_KT = IN_F // GROUP       # 32 k-tiles (1 quant group per k-tile)
KQ0, KQ1 = 8, 24           # uneven K split: short head, long overlapped body
KF8 = 6                    # of KQ1: k-tiles 8..13 run as fp8 DoubleRow pairs
KQF = KQ1 - KF8            # fp16 k-tiles 16..31 in phase 1
TOK_CHUNK = 512            # tokens per x-slab DMA
N_CHUNKS = M_TOK // TOK_CHUNK  # 16
N_TT = M_TOK // 128        # 64 token tiles
Q_N, Q_P = -8.0, 7.0
LSCALE = 1024.0            # 2^10: lifts lora operands out of fp16 subnormals

_CACHE = {}


def _build():
    nc = bacc.Bacc(None, target_bir_lowering=False)
    xT_d = nc.dram_tensor("xT16", [IN_F, M_TOK], F16, kind="ExternalInput")
    xT8_d = nc.dram_tensor("xT8", [KF8 * 128, M_TOK], F8, kind="ExternalInput")
    w0T_d = nc.dram_tensor("w0T", [IN_F, OUT_SH], F32, kind="ExternalInput")
    la_d = nc.dram_tensor("lora_a", [RANK, IN_F], F32, kind="ExternalInput")
    lbT_d = nc.dram_tensor("lora_bT", [RANK, OUT_SH], F32, kind="ExternalInput")
    qsT_d = nc.dram_tensor("qscT", [N_KT, OUT_SH], F32, kind="ExternalInput")
    bias_d = nc.dram_tensor("bias", [1, OUT_SH], F32, kind="ExternalInput")
    y_d = nc.dram_tensor("y", [M_TOK, OUT_SH], F16, kind="ExternalOutput")

    with TileContext(nc) as tc:
        with (
            tc.tile_pool(name="persist", bufs=1) as persist,
            tc.tile_pool(name="yout", bufs=4) as ypool,
            tc.tile_pool(name="pdeq", bufs=2, space="PSUM") as pdeq,
            tc.tile_pool(name="pbc", bufs=1, space="PSUM") as pbc,
            tc.tile_pool(name="pmm", bufs=3, space="PSUM") as pmm,
            tc.tile_pool(name="pwarm", bufs=1, space="PSUM") as pwarm,
            tc.tile_pool(name="dram", bufs=1, space="DRAM") as dram,
        ):
            w0pool_cm = tc.tile_pool(name="w0", bufs=2)
            w0pool = w0pool_cm.__enter__()
            deq_cm = tc.tile_pool(name="deq", bufs=2)
            deq = deq_cm.__enter__()
            lora_cm = tc.tile_pool(name="lora", bufs=1)
            lorap = lora_cm.__enter__()
            # ---------- setup ----------
            # ones for the K=3 broadcast matmuls, in two row groups (0, 32)
            # so the s and r broadcasts run concurrently in the PE array
            ones64 = persist.tile([64, 128], BF16)
            nc.vector.memset(ones64[:], 1.0)
            # instant warmup: depends only on the memset, so the PE starts
            # ramping immediately instead of waiting for the first lora DMA
            warm0_ps = pwarm.tile([128, 512], F32, tag="warm")
            for _ in range(24):
                nc.tensor.matmul(warm0_ps[:, 0:128], ones64[0:3, :],
                                 ones64[0:3, :], start=True, stop=True,
                                 tile_position=(0, 0))

            setup_pool = tc.tile_pool(name="setup", bufs=1)
            setup = setup_pool.__enter__()
            # lora operands replicated at partition offsets 0 and 32 so two
            # k-tiles' K=16 delta matmuls run concurrently in PE row groups.
            # la_sb lives in the phase-0-long lora pool: its split is partly
            # deferred past the head (see below), so it must not share space
            # with the phase-0 x slabs.
            la_sb = lorap.tile([RANK + 32, IN_F], F32)
            nc.sync.dma_start(la_sb[0:RANK, :], la_d[:, :])
            nc.sync.dma_start(la_sb[32:32 + RANK, :], la_d[:, :])
            lbT_sb = setup.tile([RANK + 32, OUT_SH], F32)
            nc.sync.dma_start(lbT_sb[0:RANK, :], lbT_d[:, :])
            nc.sync.dma_start(lbT_sb[32:32 + RANK, :], lbT_d[:, :])

            # exact 2-way fp16 Dekker split of 2^10 * lora operands: the PE
            # then computes 2^20 * delta from three fp16 matmuls (h*h, h*l,
            # l*h) at 1 cycle/row instead of one fp32 matmul at 4 cycles/row.
            la_t = lorap.tile([RANK + 32, 1024], F32)

            def split16(src32, hi, lo, c0, c1):
                nc.vector.tensor_scalar(hi[:, c0:c1], src32[:, c0:c1],
                                        LSCALE, None, AluOpType.mult)
                for cc in range(c0, c1, 1024):
                    ce = min(cc + 1024, c1)
                    nc.vector.scalar_tensor_tensor(la_t[:, 0:ce - cc],
                                                   src32[:, cc:ce], LSCALE,
                                                   hi[:, cc:ce],
                                                   AluOpType.mult,
                                                   AluOpType.subtract)
                    nc.vector.tensor_copy(lo[:, cc:ce], la_t[:, 0:ce - cc])

            la_h = lorap.tile([RANK + 32, IN_F], F16)
            la_l = lorap.tile([RANK + 32, IN_F], F16)
            lb_h = lorap.tile([RANK + 32, OUT_SH], F16)
            lb_l = lorap.tile([RANK + 32, OUT_SH], F16)
            # only the head's la columns on the critical path; the remaining
            # 3072 columns split after the head dequant is queued (their
            # consumers run interleaved under phase 0, ~100us of slack)
            split16(lbT_sb, lb_h, lb_l, 0, OUT_SH)
            split16(la_sb, la_h, la_l, 0, KQ0 * 128)

            # scales: s [32, 512]; r = 1/s via reciprocal + 2 NR (0-ulp exact
            # per HW probe)
            sT32 = setup.tile([N_KT, OUT_SH], F32)
            nc.sync.dma_start(sT32[:], qsT_d[:, :])
            r32 = setup.tile([N_KT, OUT_SH], F32)
            nc.vector.reciprocal(r32[:], sT32[:])
            t32 = setup.tile([N_KT, OUT_SH], F32)
            for _ in range(2):
                nc.vector.tensor_tensor(t32[:], sT32[:], r32[:], AluOpType.mult)
                nc.vector.tensor_scalar(t32[:], t32[:], -1.0, 2.0,
                                        AluOpType.mult, AluOpType.add)
                nc.vector.tensor_tensor(r32[:], r32[:], t32[:], AluOpType.mult)

            # exact 3-way bf16 splits of s and r: hi/mid/lo are disjoint 8-bit
            # chunks of the fp32 mantissa, so a K=3 bf16 matmul against ones
            # rebuilds the fp32 value exactly in PSUM at 1 cycle/row (a plain
            # fp32 broadcast matmul costs 4 cycles/row).
            def split3(src32, nm):
                hi = setup.tile([N_KT, OUT_SH], BF16, name=f"{nm}_hi")
                nc.vector.tensor_copy(hi[:], src32[:])
                t1 = setup.tile([N_KT, OUT_SH], F32, name=f"{nm}_t1")
                nc.vector.tensor_tensor(t1[:], src32[:], hi[:],
                                        AluOpType.subtract)
                mid = setup.tile([N_KT, OUT_SH], BF16, name=f"{nm}_mid")
                nc.vector.tensor_copy(mid[:], t1[:])
                nc.vector.tensor_tensor(t1[:], t1[:], mid[:],
                                        AluOpType.subtract)
                lo = setup.tile([N_KT, OUT_SH], BF16, name=f"{nm}_lo")
                nc.vector.tensor_copy(lo[:], t1[:])
                d3 = dram.tile([3, N_KT, OUT_SH], BF16, name=f"{nm}_d3")
                nc.sync.dma_start(d3[0], hi[:])
                nc.sync.dma_start(d3[1], mid[:])
                nc.sync.dma_start(d3[2], lo[:])
                return d3

            s3_dram = split3(sT32, "s3")
            r3_dram = split3(r32, "r3")

            # bias row staged for a later broadcast (after the head)
            biasT_sb = setup.tile([1, OUT_SH], F32)
            nc.sync.dma_start(biasT_sb[:], bias_d[:, :])
            biasT16 = persist.tile([1, OUT_SH], BF16)
            nc.vector.tensor_copy(biasT16[:], biasT_sb[:])
            setup_pool.__exit__(None, None, None)

            # fp16 phase-0 partial sums, one [128, OUT_SH] slab per token tile
            y16 = persist.tile([128, N_TT, OUT_SH], F16)

            w0T_r = w0T_d.rearrange("(kt p) o -> p kt o", p=128)
            xT_r = xT_d.rearrange("(kt p) m -> p kt m", p=128)
            xT8_r = xT8_d.rearrange("(kt p) m -> p kt m", p=128)
            y_r = y_d.rearrange("(n p) o -> n p o", p=128)

            wt0 = persist.tile([128, KQ0, OUT_SH], F16)
            wt8 = persist.tile([128, KF8, OUT_SH], F8)
            wt1 = persist.tile([128, KQF, OUT_SH], F16)
            # phase-1 first-tokens prefetch slabs (see phase-0 loop)
            xpre8 = persist.tile([128, KF8, 256], F8)
            xpre = persist.tile([128, KQF, 256], F16)

            def deq_dma(k0):
                """DMA w0 + scale/recip split rows for k-tiles k0, k0+1.
                s rows land at partitions 0-2, r rows at 32-34 so the two
                broadcast matmuls run in different PE row groups."""
                w0_sb = w0pool.tile([128, 2, OUT_SH], F32, tag="w0")
                nc.sync.dma_start(w0_sb[:], w0T_r[:, k0:k0 + 2, :])
                srrow = deq.tile([35, 2, OUT_SH], BF16, tag="srrow")
                nc.sync.dma_start(srrow[0:3], s3_dram[:, k0:k0 + 2, :])
                nc.sync.dma_start(srrow[32:35], r3_dram[:, k0:k0 + 2, :])
                return w0_sb, srrow

            def deq_k(wtq, k, kl, ki, bufs):
                """Dequantize k-tile k into wtq[:, kl, :]. Odd k-tiles use
                PE row group 32 so their small matmuls overlap the even
                k-tile's in the array."""
                w0_sb, srrow = bufs
                ks = slice(k * 128, (k + 1) * 128)
                ro = 32 * (k % 2)
                rr = slice(ro + 0, ro + RANK)
                # 2^20 * lora delta^T via 3 exact fp16 matmuls (K=16)
                d_ps = pdeq.tile([128, OUT_SH], F32, tag="dps")
                nc.tensor.matmul(d_ps[:], la_h[rr, ks], lb_h[rr, :],
                                 start=True, stop=False,
                                 tile_position=(ro, 0))
                nc.tensor.matmul(d_ps[:], la_h[rr, ks], lb_l[rr, :],
                                 start=False, stop=False,
                                 tile_position=(ro, 0))
                nc.tensor.matmul(d_ps[:], la_l[rr, ks], lb_h[rr, :],
                                 start=False, stop=True,
                                 tile_position=(ro, 0))
                # broadcast scale/recip rows k to 128 partitions via K=3
                # bf16 matmuls (exact fp32 reconstruction in PSUM), s in row
                # group 0 and r in row group 32 (concurrent)
                s_ps = pbc.tile([128, OUT_SH], F32, tag="sps")
                nc.tensor.matmul(s_ps[:], ones64[0:3, :], srrow[0:3, ki, :],
                                 start=True, stop=True,
                                 tile_position=(0, 0))
                r_ps = pbc.tile([128, OUT_SH], F32, tag="rps")
                nc.tensor.matmul(r_ps[:], ones64[32:35, :],
                                 srrow[32:35, ki, :],
                                 start=True, stop=True,
                                 tile_position=(32, 0))
                # w = 2^-20 * d + w0  (exact fp32)
                v = deq.tile([128, OUT_SH], F32, tag="v")
                nc.vector.scalar_tensor_tensor(v[:], d_ps[:], LSCALE ** -2,
                                               w0_sb[:, ki, :],
                                               AluOpType.mult, AluOpType.add)
                # v = w * (1/s)
                nc.vector.tensor_tensor(v[:], v[:], r_ps[:], AluOpType.mult)
                # clip to [-8, 7]
                nc.vector.tensor_scalar(v[:], v[:], Q_N, Q_P,
                                        AluOpType.max, AluOpType.min)
                # round half-to-even
                nc.vector.tensor_scalar(v[:], v[:], MAGIC, MAGIC,
                                        AluOpType.add, AluOpType.subtract)
                # w_q = q * s, cast to the slab dtype (fp16, or fp8 for
                # the DoubleRow k-tiles)
                nc.vector.tensor_tensor(wtq[:, kl, :], v[:], s_ps[:],
                                        AluOpType.mult)

            # warmup matmuls: keep the PE continuously busy through the
            # DVE-bound head so the HAM clock ramp reaches (and holds) the
            # full 2.4 GHz p-state before the big GEMM starts
            warm_ps = pwarm.tile([128, OUT_SH], F32, tag="warm")

            def warm(n):
                for _ in range(n):
                    nc.tensor.matmul(warm_ps[:], la_h[0:RANK, 0:128],
                                     lb_h[0:RANK, :],
                                     start=True, stop=True,
                                     tile_position=(0, 0))

            # ---------- phase-0 k-tiles dequant (head) ----------
            for pair in range(KQ0 // 2):
                bufs = deq_dma(2 * pair)
                deq_k(wt0, 2 * pair, 2 * pair, 0, bufs)
                warm(4)
                deq_k(wt0, 2 * pair + 1, 2 * pair + 1, 1, bufs)
                warm(4)

            # bias broadcast tile [128, OUT_SH] fp16 (|bias| ~ 0.01: tiny)
            bias_ps = pbc.tile([128, OUT_SH], F32, tag="sps")
            nc.tensor.matmul(bias_ps[:], ones64[0:1, :], biasT16[:],
                             start=True, stop=True)
            bias_bc = persist.tile([128, OUT_SH], F16)
            nc.vector.tensor_copy(bias_bc[:], bias_ps[:])
            # long final burst: >3us continuous PE busy flips the HAM clock
            # to the full p-state before the first GEMM chunk
            warm(16)

            # ---------- phase 0: GEMM k 0..7, dequant k 8..31 underneath ----
            xp0_cm = tc.tile_pool(name="xp0", bufs=3)
            xp0 = xp0_cm.__enter__()
            deq_bufs = None
            for c in range(N_CHUNKS):
                xs = xp0.tile([128, KQ0, TOK_CHUNK], F16, tag="xs0")
                nc.sync.dma_start(
                    xs[:],
                    xT_r[:, 0:KQ0, c * TOK_CHUNK:(c + 1) * TOK_CHUNK])
                for t in range(TOK_CHUNK // 128):
                    y_ps = pmm.tile([128, OUT_SH], F32, tag="yps")
                    for j in range(KQ0):
                        nc.tensor.matmul(y_ps[:],
                                         xs[:, j, t * 128:(t + 1) * 128],
                                         wt0[:, j, :],
                                         start=(j == 0), stop=(j == KQ0 - 1))
                    tt = c * 4 + t
                    nc.vector.tensor_tensor(y16[:, tt, :], y_ps[:],
                                            bias_bc[:], AluOpType.add)
                    # chunks 0..2: finish the la split in pieces (its bulk
                    # would otherwise delay chunk 0's PSUM drains on the DVE
                    # queue and stall the PE on bank reuse)
                    if c < 3 and t == 3:
                        cc = KQ0 * 128 + 1024 * c
                        split16(la_sb, la_h, la_l, cc, cc + 1024)
                    # interleave phase-1 dequant: 12 pairs over chunks 3..14
                    # (k 8..15 land in the fp8 slab, k 16..31 in the fp16 one)
                    if 3 <= c < 15 and t == 0:
                        k0 = KQ0 + 2 * (c - 3)
                        w_t, kl = ((wt8, k0 - KQ0) if k0 < KQ0 + KF8
                                   else (wt1, k0 - KQ0 - KF8))
                        deq_bufs = deq_dma(k0)
                        deq_k(w_t, k0, kl, 0, deq_bufs)
                    if 3 <= c < 15 and t == 2:
                        k1 = KQ0 + 2 * (c - 3) + 1
                        w_t, kl = ((wt8, k1 - KQ0) if k1 < KQ0 + KF8
                                   else (wt1, k1 - KQ0 - KF8))
                        deq_k(w_t, k1, kl, 1, deq_bufs)
                    # prefetch phase 1's first 128 tokens into a persistent
                    # tile: phase 1's pooled x slabs reuse phase-0 SBUF space
                    # and their first DMA must wait for phase 0 to finish;
                    # this keeps the PE fed across the phase boundary
                    if c == 13 and t == 0:
                        nc.sync.dma_start(xpre8[:],
                                          xT8_r[:, :, 0:256])
                        nc.sync.dma_start(xpre[:],
                                          xT_r[:, KQ0 + KF8:N_KT, 0:256])

            # ---------- phase 1: GEMM k 8..31, drain + emit y ----------
            xp0_cm.__exit__(None, None, None)
            lora_cm.__exit__(None, None, None)
            deq_cm.__exit__(None, None, None)
            w0pool_cm.__exit__(None, None, None)
            xp1_cm = tc.tile_pool(name="xp1", bufs=3)
            xp1 = xp1_cm.__enter__()
            # token chunks: 256 prefetched + 256 + 15*512 = 8192
            p1_chunks = [(0, 256), (256, 256)]
            p1_chunks += [(512 * i, 512) for i in range(1, N_CHUNKS)]
            for start, ntok in p1_chunks:
                if start == 0:
                    xs8, xs = xpre8, xpre
                else:
                    xs8 = xp1.tile([128, KF8, TOK_CHUNK], F8, tag="xs8")
                    nc.sync.dma_start(
                        xs8[:, :, 0:ntok],
                        xT8_r[:, :, start:start + ntok])
                    xs = xp1.tile([128, KQF, TOK_CHUNK], F16, tag="xs1")
                    nc.sync.dma_start(
                        xs[:, :, 0:ntok],
                        xT_r[:, KQ0 + KF8:N_KT, start:start + ntok])
                for t in range(ntok // 128):
                    y_ps = pmm.tile([128, OUT_SH], F32, tag="yps")
                    # k 8..15: four fp8 DoubleRow matmuls, each contracting
                    # two interleaved k-tiles at 2 fp8 weights per PE cell
                    for p in range(KF8 // 2):
                        nc.tensor.matmul(
                            y_ps[:],
                            xs8[:, 2 * p:2 * p + 2, t * 128:(t + 1) * 128],
                            wt8[:, 2 * p:2 * p + 2, :],
                            start=(p == 0), stop=False,
                            perf_mode=mybir.MatmulPerfMode.DoubleRow)
                    # k 16..31: fp16
                    for j in range(KQF):
                        nc.tensor.matmul(y_ps[:],
                                         xs[:, j, t * 128:(t + 1) * 128],
                                         wt1[:, j, :],
                                         start=False, stop=(j == KQF - 1))
                    tt = start // 128 + t
                    yo = ypool.tile([128, OUT_SH], F16, tag="yo")
                    nc.vector.tensor_tensor(yo[:], y_ps[:], y16[:, tt, :],
                                            AluOpType.add)
                    nc.sync.dma_start(y_r[tt], yo[:])
            xp1_cm.__exit__(None, None, None)
    nc.compile()
    return nc


def _make_in_maps(x, w0, lora_a, lora_b, q_scale, bias):
    # host-side layout marshalling (no arithmetic beyond the fp16 cast of x,
    # which is the kernel's chosen input precision for the tensor engine)
    import ml_dtypes
    x = np.ascontiguousarray(np.asarray(x, dtype=np.float32))
    xT = np.ascontiguousarray(x.reshape(M_TOK, IN_F).T)
    xT16 = xT.astype(np.float16)
    xT8 = np.ascontiguousarray(xT[KQ0 * 128:(KQ0 + KF8) * 128, :]).astype(
        ml_dtypes.float8_e4m3)
    w0T = np.ascontiguousarray(np.asarray(w0, dtype=np.float32).T)
    lbT = np.ascontiguousarray(np.asarray(lora_b, dtype=np.float32).T)
    qs2 = np.asarray(q_scale, dtype=np.float32).reshape(OUT_F, N_KT)
    bias = np.asarray(bias, dtype=np.float32)
    lora_a = np.ascontiguousarray(np.asarray(lora_a, dtype=np.float32))
    in_maps = []
    for c in range(N_CORES):
        sl = slice(c * OUT_SH, (c + 1) * OUT_SH)
        in_maps.append({
            "xT16": xT16,
            "xT8": xT8,
            "w0T": np.ascontiguousarray(w0T[:, sl]),
            "lora_a": lora_a,
            "lora_bT": np.ascontiguousarray(lbT[:, sl]),
            "qscT": np.ascontiguousarray(qs2[sl].T),
            "bias": np.ascontiguousarray(bias[sl]).reshape(1, OUT_SH),
        })
    return in_maps


def kernel(x, w0, lora_a, lora_b, q_scale, bias):
    if "nc" not in _CACHE:
        _CACHE["nc"] = _build()
    in_maps = _make_in_maps(x, w0, lora_a, lora_b, q_scale, bias)
    res = run_bass_kernel_spmd(_CACHE["nc"], in_maps,
                               core_ids=list(range(N_CORES)))
    y = np.concatenate([res.results[c]["y"].astype(np.float32)
                        for c in range(N_CORES)], axis=1)
    return y.reshape(B, S, OUT_F)


def timed_run(inputs):
    """Profiled run for test.py: returns max-core HW exec time in ns."""
    if "nc" not in _CACHE:
        _CACHE["nc"] = _build()
    in_maps = _make_in_maps(**inputs)
    res = run_bass_kernel_spmd(
        _CACHE["nc"], in_maps, core_ids=list(range(N_CORES)),
        trace=True, trace_cores=[0])
    if res.instructions_and_trace:
        insts, path = res.instructions_and_trace
        print("trace path:", path)
        if insts:
            t0 = min(i.timestamp for i in insts)
            t1 = max(i.end_timestamp for i in insts)
            span = t1 - t0
            from collections import defaultdict, Counter
            busy = defaultdict(int)
            cnt = defaultdict(int)
            for i in insts:
                busy[i.engine] += i.duration
                cnt[i.engine] += 1
            print(f"span: {span} ns")
            for e in sorted(busy, key=lambda e: -busy[e]):
                print(f"  {e:>12}: busy {busy[e]:>9} ns ({100.0*busy[e]/span:5.1f}%)"
                      f"  n={cnt[e]}")
            pe = sorted((i for i in insts if i.engine == "TensorMatrix"),
                        key=lambda i: i.timestamp)
            if pe:
                durs = np.array([i.duration for i in pe])
                print("PE dur histogram:",
                      Counter((durs // 50 * 50).tolist()).most_common(10))
                gaps = np.array([b.timestamp - a.end_timestamp
                                 for a, b in zip(pe, pe[1:])])
                gaps = gaps[gaps > 0]
                print(f"PE gaps>0: n={len(gaps)} total={gaps.sum()} "
                      f"max={gaps.max() if len(gaps) else 0}")
                print(f"PE first inst at t+{pe[0].timestamp - t0}, "
                      f"last ends at t+{pe[-1].end_timestamp - t0}")
    return res.exec_time_ns


# revision 50
# speedup vs baseline: 1.0276x; 1.0276x over previous
"""Trainium2 Bass kernel for L4Q quantized linear (LoRA + group fake-quant + GEMM).

Computation (per reference):
    w   = w0 + lora_b @ lora_a                      # [4096, 4096]
    w_q = round(clip(w/s, -8, 7)) * s               # group-wise (groups of 128 along in)
    y   = x @ w_q.T + bias                          # x: [4, 2048, 4096]

Sharding: column-parallel over out_features across 8 cores (512 outs/core).
x is replicated (pre-transposed + fp16-cast on host); each core computes
y[:, :, c*512:(c+1)*512] and the host concatenates.

Numeric strategy:
  - dequant runs in ~fp32-exact arithmetic on-device: the K=16 LoRA delta via
    an exact 2-way fp16 Dekker split of (2^10 * lora) re-scaled back with an
    exact 2^-20 fused multiply; per-group 1/s via NR-refined reciprocal;
    scale/recip rows broadcast to 128 partitions via K=3 bf16 matmuls whose
    three rows are disjoint 8-bit chunks of the fp32 mantissa (exact fp32
    reconstruction in PSUM at bf16 matmul speed); DVE IEEE fp32 elementwise
    with magic-number round-half-even. Quantization decisions match the fp32
    reference to ~1 ulp.
  - the big GEMM runs in fp16 (11-bit mantissa) with fp32 PSUM accumulation;
    y is emitted fp16 (|y| <= ~11, so ~2.5e-3 absolute) and widened on host.

Pipelining: K is split unevenly into 8 + 24 k-tiles. The 8-tile slice is
dequantized up front (short head); the 24-tile slice's dequant (DVE-heavy)
runs interleaved under the first GEMM phase so the PE stays continuously busy
at the warm 2.4 GHz p-state. Phase-0 partial sums (+bias) park in an fp16
SBUF accumulator; phase 1 adds them during its PSUM drain.
"""
import numpy as np

import concourse.bass as bass
import concourse.bacc as bacc
import concourse.mybir as mybir
from concourse.tile import TileContext
from concourse.bass_utils import run_bass_kernel_spmd
from concourse.alu_op_type import AluOpType

F32 = mybir.dt.float32
F16 = mybir.dt.float16
BF16 = mybir.dt.bfloat16
F8 = mybir.dt.float8e4
MAGIC = 12582912.0  # 1.5 * 2**23: forces round-to-nearest-even at integer granularity

N_CORES = 8
IN_F = 4096
OUT_F = 4096
RANK = 16
B, S = 4, 2048
M_TOK = B * S            # 8192 tokens
OUT_SH = OUT_F // N_CORES  # 512 out features per core
GROUP = 128
N_KT = IN_F // GROUP       # 32 k-tiles (1 quant group per k-tile)
KQ0, KQ1 = 8, 24           # uneven K split: short head, long overlapped body
KF8 = 8                    # of KQ1: k-tiles 8..15 run as fp8 DoubleRow pairs
KQF = KQ1 - KF8            # fp16 k-tiles 16..31 in phase 1
TOK_CHUNK = 512            # tokens per x-slab DMA
N_CHUNKS = M_TOK // TOK_CHUNK  # 16
N_TT = M_TOK // 128        # 64 token tiles
Q_N, Q_P = -8.0, 7.0
LSCALE = 1024.0            # 2^10: lifts lora operands out of fp16 subnormals

_CACHE = {}


def _build():
    nc = bacc.Bacc(None, target_bir_lowering=False)
    xT_d = nc.dram_tensor("xT16", [IN_F, M_TOK], F16, kind="ExternalInput")
    xT8_d = nc.dram_tensor("xT8", [KF8 * 128, M_TOK], F8, kind="ExternalInput")
    w0T_d = nc.dram_tensor("w0T", [IN_F, OUT_SH], F32, kind="ExternalInput")
    la_d = nc.dram_tensor("lora_a", [RANK, IN_F], F32, kind="ExternalInput")
    lbT_d = nc.dram_tensor("lora_bT", [RANK, OUT_SH], F32, kind="ExternalInput")
    qsT_d = nc.dram_tensor("qscT", [N_KT, OUT_SH], F32, kind="ExternalInput")
    bias_d = nc.dram_tensor("bias", [1, OUT_SH], F32, kind="ExternalInput")
    y_d = nc.dram_tensor("y", [M_TOK, OUT_SH], F16, kind="ExternalOutput")

    with TileContext(nc) as tc:
        with (
            tc.tile_pool(name="persist", bufs=1) as persist,
            tc.tile_pool(name="yout", bufs=4) as ypool,
            tc.tile_pool(name="pdeq", bufs=2, space="PSUM") as pdeq,
            tc.tile_pool(name="pbc", bufs=1, space="PSUM") as pbc,
            tc.tile_pool(name="pmm", bufs=3, space="PSUM") as pmm,
            tc.tile_pool(name="pwarm", bufs=1, space="PSUM") as pwarm,
            tc.tile_pool(name="dram", bufs=1, space="DRAM") as dram,
        ):
            w0pool_cm = tc.tile_pool(name="w0", bufs=2)
            w0pool = w0pool_cm.__enter__()
            deq_cm = tc.tile_pool(name="deq", bufs=2)
            deq = deq_cm.__enter__()
            lora_cm = tc.tile_pool(name="lora", bufs=1)
            lorap = lora_cm.__enter__()
            # ---------- setup ----------
            # ones for the K=3 broadcast matmuls, in two row groups (0, 32)
            # so the s and r broadcasts run concurrently in the PE array
            ones64 = persist.tile([64, 128], BF16)
            nc.vector.memset(ones64[:], 1.0)
            # instant warmup: depends only on the memset, so the PE starts
            # ramping immediately instead of waiting for the first lora DMA
            warm0_ps = pwarm.tile([128, 512], F32, tag="warm")
            for _ in range(24):
                nc.tensor.matmul(warm0_ps[:, 0:128], ones64[0:3, :],
                                 ones64[0:3, :], start=True, stop=True,
                                 tile_position=(0, 0))

            setup_pool = tc.tile_pool(name="setup", bufs=1)
            setup = setup_pool.__enter__()
            # lora operands replicated at partition offsets 0 and 32 so two
            # k-tiles' K=16 delta matmuls run concurrently in PE row groups.
            # la_sb lives in the phase-0-long lora pool: its split is partly
            # deferred past the head (see below), so it must not share space
            # with the phase-0 x slabs.
            la_sb = lorap.tile([RANK + 32, IN_F], F32)
            nc.sync.dma_start(la_sb[0:RANK, :], la_d[:, :])
            nc.sync.dma_start(la_sb[32:32 + RANK, :], la_d[:, :])
            lbT_sb = setup.tile([RANK + 32, OUT_SH], F32)
            nc.sync.dma_start(lbT_sb[0:RANK, :], lbT_d[:, :])
            nc.sync.dma_start(lbT_sb[32:32 + RANK, :], lbT_d[:, :])

            # exact 2-way fp16 Dekker split of 2^10 * lora operands: the PE
            # then computes 2^20 * delta from three fp16 matmuls (h*h, h*l,
            # l*h) at 1 cycle/row instead of one fp32 matmul at 4 cycles/row.
            la_t = lorap.tile([RANK + 32, 1024], F32)

            def split16(src32, hi, lo, c0, c1):
                nc.vector.tensor_scalar(hi[:, c0:c1], src32[:, c0:c1],
                                        LSCALE, None, AluOpType.mult)
                for cc in range(c0, c1, 1024):
                    ce = min(cc + 1024, c1)
                    nc.vector.scalar_tensor_tensor(la_t[:, 0:ce - cc],
                                                   src32[:, cc:ce], LSCALE,
                                                   hi[:, cc:ce],
                                                   AluOpType.mult,
                                                   AluOpType.subtract)
                    nc.vector.tensor_copy(lo[:, cc:ce], la_t[:, 0:ce - cc])

            la_h = lorap.tile([RANK + 32, IN_F], F16)
            la_l = lorap.tile([RANK + 32, IN_F], F16)
            lb_h = lorap.tile([RANK + 32, OUT_SH], F16)
            lb_l = lorap.tile([RANK + 32, OUT_SH], F16)
            # only the head's la columns on the critical path; the remaining
            # 3072 columns split after the head dequant is queued (their
            # consumers run interleaved under phase 0, ~100us of slack)
            split16(lbT_sb, lb_h, lb_l, 0, OUT_SH)
            split16(la_sb, la_h, la_l, 0, KQ0 * 128)

            # scales: s [32, 512]; r = 1/s via reciprocal + 2 NR (0-ulp exact
            # per HW probe)
            sT32 = setup.tile([N_KT, OUT_SH], F32)
            nc.sync.dma_start(sT32[:], qsT_d[:, :])
            r32 = setup.tile([N_KT, OUT_SH], F32)
            nc.vector.reciprocal(r32[:], sT32[:])
            t32 = setup.tile([N_KT, OUT_SH], F32)
            for _ in range(2):
                nc.vector.tensor_tensor(t32[:], sT32[:], r32[:], AluOpType.mult)
                nc.vector.tensor_scalar(t32[:], t32[:], -1.0, 2.0,
                                        AluOpType.mult, AluOpType.add)
                nc.vector.tensor_tensor(r32[:], r32[:], t32[:], AluOpType.mult)

            # exact 3-way bf16 splits of s and r: hi/mid/lo are disjoint 8-bit
            # chunks of the fp32 mantissa, so a K=3 bf16 matmul against ones
            # rebuilds the fp32 value exactly in PSUM at 1 cycle/row (a plain
            # fp32 broadcast matmul costs 4 cycles/row).
            def split3(src32, nm):
                hi = setup.tile([N_KT, OUT_SH], BF16, name=f"{nm}_hi")
                nc.vector.tensor_copy(hi[:], src32[:])
                t1 = setup.tile([N_KT, OUT_SH], F32, name=f"{nm}_t1")
                nc.vector.tensor_tensor(t1[:], src32[:], hi[:],
                                        AluOpType.subtract)
                mid = setup.tile([N_KT, OUT_SH], BF16, name=f"{nm}_mid")
                nc.vector.tensor_copy(mid[:], t1[:])
                nc.vector.tensor_tensor(t1[:], t1[:], mid[:],
                                        AluOpType.subtract)
                lo = setup.tile([N_KT, OUT_SH], BF16, name=f"{nm}_lo")
                nc.vector.tensor_copy(lo[:], t1[:])
                d3 = dram.tile([3, N_KT, OUT_SH], BF16, name=f"{nm}_d3")
                nc.sync.dma_start(d3[0], hi[:])
                nc.sync.dma_start(d3[1], mid[:])
                nc.sync.dma_start(d3[2], lo[:])
                return d3

            s3_dram = split3(sT32, "s3")
            r3_dram = split3(r32, "r3")

            # bias row staged for a later broadcast (after the head)
            biasT_sb = setup.tile([1, OUT_SH], F32)
            nc.sync.dma_start(biasT_sb[:], bias_d[:, :])
            biasT16 = persist.tile([1, OUT_SH], BF16)
            nc.vector.tensor_copy(biasT16[:], biasT_sb[:])
            setup_pool.__exit__(None, None, None)

            # fp16 phase-0 partial sums, one [128, OUT_SH] slab per token tile
            y16 = persist.tile([128, N_TT, OUT_SH], F16)

            w0T_r = w0T_d.rearrange("(kt p) o -> p kt o", p=128)
            xT_r = xT_d.rearrange("(kt p) m -> p kt m", p=128)
            xT8_r = xT8_d.rearrange("(kt p) m -> p kt m", p=128)
            y_r = y_d.rearrange("(n p) o -> n p o", p=128)

            wt0 = persist.tile([128, KQ0, OUT_SH], F16)
            wt8 = persist.tile([128, KF8, OUT_SH], F8)
            wt1 = persist.tile([128, KQF, OUT_SH], F16)
            # phase-1 first-tokens prefetch slabs (see phase-0 loop)
            xpre8 = persist.tile([128, KF8, 256], F8)
            xpre = persist.tile([128, KQF, 256], F16)

            def deq_dma(k0):
                """DMA w0 + scale/recip split rows for k-tiles k0, k0+1.
                s rows land at partitions 0-2, r rows at 32-34 so the two
                broadcast matmuls run in different PE row groups."""
                w0_sb = w0pool.tile([128, 2, OUT_SH], F32, tag="w0")
                nc.sync.dma_start(w0_sb[:], w0T_r[:, k0:k0 + 2, :])
                srrow = deq.tile([35, 2, OUT_SH], BF16, tag="srrow")
                nc.sync.dma_start(srrow[0:3], s3_dram[:, k0:k0 + 2, :])
                nc.sync.dma_start(srrow[32:35], r3_dram[:, k0:k0 + 2, :])
                return w0_sb, srrow

            def deq_k(wtq, k, kl, ki, bufs):
                """Dequantize k-tile k into wtq[:, kl, :]. Odd k-tiles use
                PE row group 32 so their small matmuls overlap the even
                k-tile's in the array."""
                w0_sb, srrow = bufs
                ks = slice(k * 128, (k + 1) * 128)
                ro = 32 * (k % 2)
                rr = slice(ro + 0, ro + RANK)
                # 2^20 * lora delta^T via 3 exact fp16 matmuls (K=16)
                d_ps = pdeq.tile([128, OUT_SH], F32, tag="dps")
                nc.tensor.matmul(d_ps[:], la_h[rr, ks], lb_h[rr, :],
                                 start=True, stop=False,
                                 tile_position=(ro, 0))
                nc.tensor.matmul(d_ps[:], la_h[rr, ks], lb_l[rr, :],
                                 start=False, stop=False,
                                 tile_position=(ro, 0))
                nc.tensor.matmul(d_ps[:], la_l[rr, ks], lb_h[rr, :],
                                 start=False, stop=True,
                                 tile_position=(ro, 0))
                # broadcast scale/recip rows k to 128 partitions via K=3
                # bf16 matmuls (exact fp32 reconstruction in PSUM), s in row
                # group 0 and r in row group 32 (concurrent)
                s_ps = pbc.tile([128, OUT_SH], F32, tag="sps")
                nc.tensor.matmul(s_ps[:], ones64[0:3, :], srrow[0:3, ki, :],
                                 start=True, stop=True,
                                 tile_position=(0, 0))
                r_ps = pbc.tile([128, OUT_SH], F32, tag="rps")
                nc.tensor.matmul(r_ps[:], ones64[32:35, :],
                                 srrow[32:35, ki, :],
                                 start=True, stop=True,
                                 tile_position=(32, 0))
                # w = 2^-20 * d + w0  (exact fp32)
                v = deq.tile([128, OUT_SH], F32, tag="v")
                nc.vector.scalar_tensor_tensor(v[:], d_ps[:], LSCALE ** -2,
                                               w0_sb[:, ki, :],
                                               AluOpType.mult, AluOpType.add)
                # v = w * (1/s)
                nc.vector.tensor_tensor(v[:], v[:], r_ps[:], AluOpType.mult)
                # clip to [-8, 7]
                nc.vector.tensor_scalar(v[:], v[:], Q_N, Q_P,
                                        AluOpType.max, AluOpType.min)
                # round half-to-even
                nc.vector.tensor_scalar(v[:], v[:], MAGIC, MAGIC,
                                        AluOpType.add, AluOpType.subtract)
                # w_q = q * s, cast to the slab dtype (fp16, or fp8 for
                # the DoubleRow k-tiles)
                nc.vector.tensor_tensor(wtq[:, kl, :], v[:], s_ps[:],
                                        AluOpType.mult)

            # warmup matmuls: keep the PE continuously busy through the
            # DVE-bound head so the HAM clock ramp reaches (and holds) the
            # full 2.4 GHz p-state before the big GEMM starts
            warm_ps = pwarm.tile([128, OUT_SH], F32, tag="warm")

            def warm(n):
                for _ in range(n):
                    nc.tensor.matmul(warm_ps[:], la_h[0:RANK, 0:128],
                                     lb_h[0:RANK, :],
                                     start=True, stop=True,
                                     tile_position=(0, 0))

            # ---------- phase-0 k-tiles dequant (head) ----------
            for pair in range(KQ0 // 2):
                bufs = deq_dma(2 * pair)
                deq_k(wt0, 2 * pair, 2 * pair, 0, bufs)
                warm(4)
                deq_k(wt0, 2 * pair + 1, 2 * pair + 1, 1, bufs)
                warm(4)

            # deferred la split for k-tiles 8..31 (consumed by the
            # phase-0-interleaved dequant, which has plenty of slack)
            split16(la_sb, la_h, la_l, KQ0 * 128, IN_F)

            # bias broadcast tile [128, OUT_SH] fp16 (|bias| ~ 0.01: tiny)
            bias_ps = pbc.tile([128, OUT_SH], F32, tag="sps")
            nc.tensor.matmul(bias_ps[:], ones64[0:1, :], biasT16[:],
                             start=True, stop=True)
            bias_bc = persist.tile([128, OUT_SH], F16)
            nc.vector.tensor_copy(bias_bc[:], bias_ps[:])
            # long final burst: >3us continuous PE busy flips the HAM clock
            # to the full p-state before the first GEMM chunk
            warm(16)

            # ---------- phase 0: GEMM k 0..7, dequant k 8..31 underneath ----
            xp0_cm = tc.tile_pool(name="xp0", bufs=3)
            xp0 = xp0_cm.__enter__()
            deq_bufs = None
            for c in range(N_CHUNKS):
                xs = xp0.tile([128, KQ0, TOK_CHUNK], F16, tag="xs0")
                nc.sync.dma_start(
                    xs[:],
                    xT_r[:, 0:KQ0, c * TOK_CHUNK:(c + 1) * TOK_CHUNK])
                for t in range(TOK_CHUNK // 128):
                    y_ps = pmm.tile([128, OUT_SH], F32, tag="yps")
                    for j in range(KQ0):
                        nc.tensor.matmul(y_ps[:],
                                         xs[:, j, t * 128:(t + 1) * 128],
                                         wt0[:, j, :],
                                         start=(j == 0), stop=(j == KQ0 - 1))
                    tt = c * 4 + t
                    nc.vector.tensor_tensor(y16[:, tt, :], y_ps[:],
                                            bias_bc[:], AluOpType.add)
                    # interleave phase-1 dequant: 12 pairs over chunks 0..11
                    # (k 8..15 land in the fp8 slab, k 16..31 in the fp16 one)
                    if c < 12 and t == 0:
                        k0 = KQ0 + 2 * c
                        w_t, kl = ((wt8, k0 - KQ0) if k0 < KQ0 + KF8
                                   else (wt1, k0 - KQ0 - KF8))
                        deq_bufs = deq_dma(k0)
                        deq_k(w_t, k0, kl, 0, deq_bufs)
                    if c < 12 and t == 2:
                        k1 = KQ0 + 2 * c + 1
                        w_t, kl = ((wt8, k1 - KQ0) if k1 < KQ0 + KF8
                                   else (wt1, k1 - KQ0 - KF8))
                        deq_k(w_t, k1, kl, 1, deq_bufs)
                    # prefetch phase 1's first 128 tokens into a persistent
                    # tile: phase 1's pooled x slabs reuse phase-0 SBUF space
                    # and their first DMA must wait for phase 0 to finish;
                    # this keeps the PE fed across the phase boundary
                    if c == 13 and t == 0:
                        nc.sync.dma_start(xpre8[:],
                                          xT8_r[:, :, 0:256])
                        nc.sync.dma_start(xpre[:],
                                          xT_r[:, KQ0 + KF8:N_KT, 0:256])

            # ---------- phase 1: GEMM k 8..31, drain + emit y ----------
            xp0_cm.__exit__(None, None, None)
            lora_cm.__exit__(None, None, None)
            deq_cm.__exit__(None, None, None)
            w0pool_cm.__exit__(None, None, None)
            xp1_cm = tc.tile_pool(name="xp1", bufs=3)
            xp1 = xp1_cm.__enter__()
            # token chunks: 256 prefetched + 256 + 15*512 = 8192
            p1_chunks = [(0, 256), (256, 256)]
            p1_chunks += [(512 * i, 512) for i in range(1, N_CHUNKS)]
            for start, ntok in p1_chunks:
                if start == 0:
                    xs8, xs = xpre8, xpre
                else:
                    xs8 = xp1.tile([128, KF8, TOK_CHUNK], F8, tag="xs8")
                    nc.sync.dma_start(
                        xs8[:, :, 0:ntok],
                        xT8_r[:, :, start:start + ntok])
                    xs = xp1.tile([128, KQF, TOK_CHUNK], F16, tag="xs1")
                    nc.sync.dma_start(
                        xs[:, :, 0:ntok],
                        xT_r[:, KQ0 + KF8:N_KT, start:start + ntok])
                for t in range(ntok // 128):
                    y_ps = pmm.tile([128, OUT_SH], F32, tag="yps")
                    # k 8..15: four fp8 DoubleRow matmuls, each contracting
                    # two interleaved k-tiles at 2 fp8 weights per PE cell
                    for p in range(KF8 // 2):
                        nc.tensor.matmul(
                            y_ps[:],
                            xs8[:, 2 * p:2 * p + 2, t * 128:(t + 1) * 128],
                            wt8[:, 2 * p:2 * p + 2, :],
                            start=(p == 0), stop=False,
                            perf_mode=mybir.MatmulPerfMode.DoubleRow)
                    # k 16..31: fp16
                    for j in range(KQF):
                        nc.tensor.matmul(y_ps[:],
                                         xs[:, j, t * 128:(t + 1) * 128],
                                         wt1[:, j, :],
                                         start=False, stop=(j == KQF - 1))
                    tt = start // 128 + t
                    yo = ypool.tile([128, OUT_SH], F16, tag="yo")
                    nc.vector.tensor_tensor(yo[:], y_ps[:], y16[:, tt, :],
                                            AluOpType.add)
                    nc.sync.dma_start(y_r[tt], yo[:])
            xp1_cm.__exit__(None, None, None)
    nc.compile()
    return nc


def _make_in_maps(x, w0, lora_a, lora_b, q_scale, bias):
    # host-side layout marshalling (no arithmetic beyond the fp16 cast of x,
    # which is the kernel's chosen input precision for the tensor engine)
    import ml_dtypes
    x = np.ascontiguousarray(np.asarray(x, dtype=np.float32))
    xT = np.ascontiguousarray(x.reshape(M_TOK, IN_F).T)
    xT16 = xT.astype(np.float16)
    xT8 = np.ascontiguousarray(xT[KQ0 * 128:(KQ0 + KF8) * 128, :]).astype(
        ml_dtypes.float8_e4m3)
    w0T = np.ascontiguousarray(np.asarray(w0, dtype=np.float32).T)
    lbT = np.ascontiguousarray(np.asarray(lora_b, dtype=np.float32).T)
    qs2 = np.asarray(q_scale, dtype=np.float32).reshape(OUT_F, N_KT)
    bias = np.asarray(bias, dtype=np.float32)
    lora_a = np.ascontiguousarray(np.asarray(lora_a, dtype=np.float32))
    in_maps = []
    for c in range(N_CORES):
        sl = slice(c * OUT_SH, (c + 1) * OUT_SH)
        in_maps.append({
            "xT16": xT16,
            "xT8": xT8,
            "w0T": np.ascontiguousarray(w0T[:, sl]),
            "lora_a": lora_a,
            "lora_bT": np.ascontiguousarray(lbT[:, sl]),
            "qscT": np.ascontiguousarray(qs2[sl].T),
            "bias": np.ascontiguousarray(bias[sl]).reshape(1, OUT_SH),
        })
    return in_maps


def kernel(x, w0, lora_a, lora_b, q_scale, bias):
    if "nc" not in _CACHE:
        _CACHE["nc"] = _build()
    in_maps = _make_in_maps(x, w0, lora_a, lora_b, q_scale, bias)
    res = run_bass_kernel_spmd(_CACHE["nc"], in_maps,
                               core_ids=list(range(N_CORES)))
    y = np.concatenate([res.results[c]["y"].astype(np.float32)
                        for c in range(N_CORES)], axis=1)
    return y.reshape(B, S, OUT_F)


def timed_run(inputs):
    """Profiled run for test.py: returns max-core HW exec time in ns."""
    if "nc" not in _CACHE:
        _CACHE["nc"] = _build()
    in_maps = _make_in_maps(**inputs)
    res = run_bass_kernel_spmd(
        _CACHE["nc"], in_maps, core_ids=list(range(N_CORES)),
        trace=True, trace_cores=[0])
    if res.instructions_and_trace:
        insts, path = res.instructions_and_trace
        print("trace path:", path)
        if insts:
            t0 = min(i.timestamp for i in insts)
            t1 = max(i.end_timestamp for i in insts)
            span = t1 - t0
            from collections import defaultdict, Counter
            busy = defaultdict(int)
            cnt = defaultdict(int)
            for i in insts:
                busy[i.engine] += i.duration
                cnt[i.engine] += 1
            print(f"span: {span} ns")
            for e in sorted(busy, key=lambda e: -busy[e]):
                print(f"  {e:>12}: busy {busy[e]:>9} ns ({100.0*busy[e]/span:5.1f}%)"
                      f"  n={cnt[e]}")
            pe = sorted((i for i in insts if i.engine == "TensorMatrix"),
                        key=lambda i: i.timestamp)
            if pe:
                durs = np.array([i.duration for i in pe])
                print("PE dur histogram:",
                      Counter((durs // 50 * 50).tolist()).most_common(10))
                gaps = np.array([b.timestamp - a.end_timestamp
                                 for a, b in zip(pe, pe[1:])])
                gaps = gaps[gaps > 0]
                print(f"PE gaps>0: n={len(gaps)} total={gaps.sum()} "
                      f"max={gaps.max() if len(gaps) else 0}")
                print(f"PE first inst at t+{pe[0].timestamp - t0}, "
                      f"last ends at t+{pe[-1].end_timestamp - t0}")
    return res.exec_time_ns
